# revision 50
# baseline (speedup 1.0000x reference)
"""AmbientReflectionNet Trainium2 kernel (8 NeuronCores, data parallel).

Reference computation (per point):
  n = l2norm(normals); v = l2norm(view_dirs)
  visible = dot(n, v) > 0
  diffuse  = visible ? MLP_d(n)              : 0   (3->256->256->256->3, ReLU)
  specular = visible ? MLP_s([n,v,rough,r0]) : 0   (8->256->256->256->3, ReLU)

Strategy:
  - Host routes only visible points (dot > 0 exactly, so no device-side mask
    is needed at all), normalizes, evaluates the tiny first layer (1% of
    FLOPs) in fp32, and packs h1 = relu(x@W0+b0) for both nets in bf16,
    feature-major, in the exact SBUF tile layout the device consumes:
    one fully contiguous 8KB/partition DMA per 1024-point tile pair.
  - Device runs the two expensive 256x256 mid layers + the 256->3 head as a
    pure bf16 matmul pipeline: per pair 32 N=512 mid matmuls at the
    216ns/MM issue roofline, plus the head as 4 concurrent column-tiled
    streams (d/s x u at PE columns 0/32/64/96). Bias+ReLU epilogues are
    FD=1024 psum->sbuf ops alternating between ScalarE and VectorE.
  - The head is software-pipelined one pair behind (emitted between the two
    mid layers) so its operands' epilogue latency is hidden and only one
    head chain sits on the kernel tail; the last pair uses per-u FD=512
    epilogues to shorten that chain.
  - A warmup matmul burst on a memset tile covers the ~9us DMA-ring boot
    and keeps the PE HAM clock gate at 8/8 from the start; weights ride in
    two slabs (dW1 first) on the sync HWDGE ring while the first input pair
    rides the ACT ring.

NOTE: matmul issue rate is layout-sensitive (216 vs ~259 ns/MM regimes,
~+27us whole-kernel). Chained AP slicing (ap[:, c][:, u]), changing pool
buffer counts, or reordering const-pool allocations have each flipped the
slow regime on. Keep rhs slices single-step and the allocation order as is.
"""

import numpy as np

import concourse.bass as bass
import concourse.mybir as mybir
import concourse.tile as tile
from concourse import bacc
from concourse.bass_utils import run_bass_kernel_spmd

NCORES = 8
P_FULL = 262144
TILE = 512
DEFAULT_NT = 32  # tiles per core (compacted); must be even
H = 256
F32 = mybir.dt.float32
BF16 = mybir.dt.bfloat16
EPS = 1e-12

_CACHE = {}


def _build(nt):
    from contextlib import ExitStack

    assert nt % 2 == 0
    nt2 = nt // 2
    ppc = nt * TILE

    nc = bacc.Bacc()

    # h1 for both nets, packed [p, pair, pfx, u, c, n]: per pair one
    # contiguous 8KB/partition transfer
    X_in = nc.declare_dram_parameter(
        "xh", [128, nt2, 2, 2, 2, TILE], BF16, isOutput=False
    )
    # weights in two slabs ordered so the very first matmuls' weights land
    # first: wslab1 [p, c, 256] = dW1;
    # wslab2 [p, c, 776] = [sW1|dW2|sW2](256 each) [dW3|sW3](4 each).
    # biases in one f32 slab [p, 10]: [dB1|sB1|dB2|sB2](2 half-cols each)
    # [b3d|b3s](1 each, partitions 0:3). Slab uploads keep the ring free
    # for inputs.
    wslab1_in = nc.declare_dram_parameter("wslab1", [128, 2, 256], BF16, isOutput=False)
    wslab2_in = nc.declare_dram_parameter("wslab2", [128, 2, 776], BF16, isOutput=False)
    bslab_in = nc.declare_dram_parameter("bslab", [128, 10], F32, isOutput=False)

    out_d = nc.declare_dram_parameter("out_d", [3, ppc], F32, isOutput=True)
    out_s = nc.declare_dram_parameter("out_s", [3, ppc], F32, isOutput=True)

    with tile.TileContext(nc) as tc, ExitStack() as ctx:
        const = ctx.enter_context(tc.tile_pool(name="const", bufs=1))
        pool_x = ctx.enter_context(tc.tile_pool(name="px", bufs=3))
        pool_h = ctx.enter_context(tc.tile_pool(name="ph", bufs=2))
        pool_o = ctx.enter_context(tc.tile_pool(name="po", bufs=3))
        ps_mm = ctx.enter_context(tc.tile_pool(name="psmm", bufs=4, space="PSUM"))

        # ---- constants (three slab DMAs, in need order) ----
        bslab = const.tile([128, 10], F32, name="bslab")
        nc.sync.dma_start(bslab, bslab_in[:, :])
        wslab1 = const.tile([128, 2, 256], BF16, name="wslab1")
        nc.sync.dma_start(wslab1, wslab1_in[:, :, :])
        wslab2 = const.tile([128, 2, 776], BF16, name="wslab2")
        nc.sync.dma_start(wslab2, wslab2_in[:, :, :])

        WOFF = {("s", 1): 0, ("d", 2): 256, ("s", 2): 512}

        def W_ap(pfx, wi, c, half):
            if (pfx, wi) == ("d", 1):
                return wslab1[:, c, half * 128 : half * 128 + 128]
            o = WOFF[pfx, wi] + half * 128
            return wslab2[:, c, o : o + 128]

        def W3_ap(pfx, c):
            o = 768 + (0 if pfx == "d" else 4)
            return wslab2[:, c, o : o + 4]

        def B_ap(pfx, wi, half):
            o = {("d", 1): 0, ("s", 1): 2, ("d", 2): 4, ("s", 2): 6}[pfx, wi] + half
            return bslab[:, o : o + 1]

        def B3_ap(pfx):
            o = 8 if pfx == "d" else 9
            return bslab[0:4, o : o + 1]

        # ---- HAM warmup: keep the PE busy from t~0 (through the ~9us DMA
        # ring boot + first transfers) so the clock gate is at 8/8 and the
        # PE queue drains right into the first real matmul. Uses a memset
        # tile so it depends on no DMA. ----
        wsrc = const.tile([128, 128], BF16, name="wsrc")
        nc.vector.memset(wsrc, 0.0)
        wps = ps_mm.tile([128, 2, TILE], F32, tag="mm", name="wps")
        for _ in range(36):
            nc.tensor.matmul(wps[:, 0, 0:128], wsrc, wsrc, start=True, stop=True)

        # bias+ReLU epilogue, alternating engines so the two per-net halves
        # drain in parallel
        def relu_epi(pfx, half, dst, psrc, bias_ap):
            if (pfx == "d") == (half == 0):
                nc.scalar.activation(
                    dst, psrc, mybir.ActivationFunctionType.Relu, bias=bias_ap
                )
            else:
                nc.vector.tensor_scalar(
                    dst, psrc, bias_ap, 0.0, mybir.AluOpType.add, mybir.AluOpType.max
                )

        def mid_layer(li, rhs_of, usplit=False):
            """One 256->256 layer for both nets; rhs_of(pfx, c, u) -> AP.

            li is the produced hidden's index (2 or 3); weights are W{li-1}.
            usplit issues per-u FD=512 epilogues across both engines so the
            results land with minimum latency (used for the last pair, whose
            head is on the kernel's tail critical path).
            """
            wi = li - 1
            hnext = {}
            for pfx in ("d", "s"):
                hnext[pfx] = pool_h.tile(
                    [128, 2, 2, TILE], BF16, tag=f"h{li}{pfx}", name=f"h{li}{pfx}"
                )
            for pi, pfx in enumerate(("d", "s")):
                for half in range(2):
                    ps = ps_mm.tile([128, 2, TILE], F32, tag="mm", name=f"ps{li}")
                    for c in range(2):
                        wap = W_ap(pfx, wi, c, half)
                        for u in range(2):
                            nc.tensor.matmul(
                                ps[:, u, :], wap, rhs_of(pfx, c, u),
                                start=(c == 0), stop=(c == 1),
                            )
                    if usplit:
                        for u in range(2):
                            relu_epi(
                                pfx, (pi + half + u) % 2,
                                hnext[pfx][:, half, u], ps[:, u, :],
                                B_ap(pfx, wi, half),
                            )
                    else:
                        relu_epi(
                            pfx, half, hnext[pfx][:, half], ps, B_ap(pfx, wi, half)
                        )
            return hnext

        def emit_l3(tp, h3):
            # head: 4 concurrent column-tiled streams, one per (net, u):
            # d-u0 @ cols 0, s-u0 @ 32, d-u1 @ 64, s-u1 @ 96
            ps3 = ps_mm.tile([100, TILE], F32, tag="mm", name="ps3")
            for c in range(2):
                for u in range(2):
                    for pi, pfx in enumerate(("d", "s")):
                        col = 64 * u + 32 * pi
                        nc.tensor.matmul(
                            ps3[col : col + 4, :], W3_ap(pfx, c), h3[pfx][:, c, u, :],
                            start=(c == 0), stop=(c == 1), tile_position=(0, col),
                        )
            osb = pool_o.tile([100, TILE], F32, tag="osb", name="osb")
            # all four head epilogues on ACT: keeps VectorE light so its
            # mid-layer ReLU epilogues land before the L2 matmuls need them
            for u in range(2):
                nc.scalar.activation(
                    osb[64 * u : 64 * u + 4], ps3[64 * u : 64 * u + 4],
                    mybir.ActivationFunctionType.Identity, bias=B3_ap("d"),
                )
                nc.scalar.activation(
                    osb[64 * u + 32 : 64 * u + 36], ps3[64 * u + 32 : 64 * u + 36],
                    mybir.ActivationFunctionType.Identity, bias=B3_ap("s"),
                )
            for u in range(2):
                for pi, outbuf in enumerate((out_d, out_s)):
                    row = 64 * u + 32 * pi
                    nc.sync.dma_start(
                        outbuf[:, (tp * 2 + u) * TILE : (tp * 2 + u + 1) * TILE],
                        osb[row : row + 3],
                    )

        h3_prev = None
        for tp in range(nt2):
            X = pool_x.tile([128, 2, 2, 2, TILE], BF16, tag="X", name="X")
            if tp == 0:
                # first pair rides the second HWDGE ring (ACT), d-half first,
                # so it streams in parallel with the weight slabs on the
                # sync ring and the first L1-d matmuls can start earliest
                # (finer chunking is a net loss: ~1-2us fixed completion
                # latency per DMA dominates small transfers)
                nc.scalar.dma_start(X[:, 0], X_in[:, 0, 0])
                nc.scalar.dma_start(X[:, 1], X_in[:, 0, 1])
            else:
                nc.sync.dma_start(X, X_in[:, tp])

            h2 = mid_layer(2, lambda pfx, c, u: X[:, 0 if pfx == "d" else 1, u, c, :])
            # previous pair's head goes here: its operands' epilogues are a
            # full layer old (no PE stall), and it stays off the kernel tail
            if h3_prev is not None:
                emit_l3(tp - 1, h3_prev)
            h3_prev = mid_layer(
                3, lambda pfx, c, u: h2[pfx][:, c, u, :], usplit=(tp == nt2 - 1)
            )
        emit_l3(nt2 - 1, h3_prev)

    nc.compile()
    return nc


def get_nc(nt=DEFAULT_NT):
    key = ("nc", nt)
    if key not in _CACHE:
        _CACHE[key] = _build(nt)
    return _CACHE[key]


def _required_nt(nv):
    """Tiles per core needed for nv compacted points (rounded up to even)."""
    nt = -(-nv // (NCORES * TILE))
    nt += nt % 2
    return max(nt, 2)


def _host_prep(inputs):
    """Visibility compaction + normalize + layer 0 + bf16 pack on host."""
    import ml_dtypes

    bf = ml_dtypes.bfloat16
    nrm = np.asarray(inputs["normals"], np.float32)
    vd = np.asarray(inputs["view_dirs"], np.float32)
    ro = np.asarray(inputs["roughness"], np.float32)
    r0 = np.asarray(inputs["r0"], np.float32)
    nn = nrm / np.maximum(np.linalg.norm(nrm, axis=1, keepdims=True), EPS)
    vv = vd / np.maximum(np.linalg.norm(vd, axis=1, keepdims=True), EPS)
    dot = np.einsum("ij,ij->i", nn, vv)
    vis_idx = np.nonzero(dot > 0)[0]

    x_d = nn[vis_idx]
    x_s = np.concatenate([nn, vv, ro, r0], axis=1)[vis_idx]
    h1 = {
        "d": np.maximum(
            x_d @ np.asarray(inputs["dW0"], np.float32)
            + np.asarray(inputs["db0"], np.float32),
            0.0,
        ),
        "s": np.maximum(
            x_s @ np.asarray(inputs["sW0"], np.float32)
            + np.asarray(inputs["sb0"], np.float32),
            0.0,
        ),
    }

    # weight slabs: wslab1 [p, c, 256] = dW1;
    # wslab2 [p, c, 776] = [sW1|dW2|sW2] [dW3|sW3](4 each)
    wslab1 = np.zeros((128, 2, 256), bf)
    wslab2 = np.zeros((128, 2, 776), bf)

    def wpack(name):
        Wm = np.asarray(inputs[name], np.float32)  # [256, 256]
        return Wm.reshape(2, 128, H).transpose(1, 0, 2).astype(bf)

    wslab1[:, :, :] = wpack("dW1")
    for i, name in enumerate(("sW1", "dW2", "sW2")):
        wslab2[:, :, i * 256 : (i + 1) * 256] = wpack(name)
    for i, pfx in enumerate(("d", "s")):
        W3 = np.asarray(inputs[f"{pfx}W3"], np.float32)  # [256, 3]
        W3p = np.concatenate([W3, np.zeros((H, 1), np.float32)], axis=1)
        wslab2[:, :, 768 + 4 * i : 772 + 4 * i] = (
            W3p.reshape(2, 128, 4).transpose(1, 0, 2).astype(bf)
        )
    # bias slab [p, 10]: [dB1|sB1|dB2|sB2](2) [b3d|b3s](1, partitions 0:3)
    bslab = np.zeros((128, 10), np.float32)
    for idx, (pfx, li) in enumerate((("d", 1), ("s", 1), ("d", 2), ("s", 2))):
        b = np.asarray(inputs[f"{pfx}b{li}"], np.float32)
        bslab[:, 2 * idx : 2 * idx + 2] = b.reshape(2, 128).T
    for i, pfx in enumerate(("d", "s")):
        bslab[0:3, 8 + i] = np.asarray(inputs[f"{pfx}b3"], np.float32)
    w = {"wslab1": wslab1, "wslab2": wslab2, "bslab": bslab}
    return vis_idx, h1, w


def make_shards(inputs, nt=DEFAULT_NT):
    """Build per-core shards; vis_idx stashed for gather_outputs."""
    import ml_dtypes

    bf = ml_dtypes.bfloat16
    vis_idx, h1, w = _host_prep(inputs)
    nv = len(vis_idx)
    need = _required_nt(nv)
    assert need <= nt, (
        f"visible points {nv} need {need} tiles/core but kernel built for {nt}"
    )
    nt2 = nt // 2
    ppc = nt * TILE
    cap = NCORES * ppc

    # pack h1 [cap, 256] -> [core, p, pair, u, c, n]
    def pack(hm):
        Hp = np.zeros((cap, H), bf)
        Hp[:nv] = hm.astype(bf)
        A = Hp.reshape(NCORES, nt2, 2, TILE, 2, 128)  # [core, t, u, n, c, p]
        return A.transpose(0, 5, 1, 2, 4, 3)  # [core, p, t, u, c, n]

    # X: [core, p, t, pfx, u, c, n]
    X = np.empty((NCORES, 128, nt2, 2, 2, 2, TILE), bf)
    X[:, :, :, 0] = pack(h1["d"])
    X[:, :, :, 1] = pack(h1["s"])

    shards = []
    for i in range(NCORES):
        m = {"xh": X[i]}
        m.update(w)
        shards.append(m)
    _CACHE["vis_idx"] = vis_idx
    _CACHE["ppc"] = ppc
    return shards


def gather_outputs(results):
    vis_idx = _CACHE["vis_idx"]
    ppc = _CACHE["ppc"]
    nv = len(vis_idx)
    diff = np.zeros((P_FULL, 3), np.float32)
    spec = np.zeros((P_FULL, 3), np.float32)
    for i in range(NCORES):
        lo = i * ppc
        hi = min(lo + ppc, nv)
        if hi <= lo:
            break
        sl = vis_idx[lo:hi]
        diff[sl] = results[i]["out_d"][:, : hi - lo].T
        spec[sl] = results[i]["out_s"][:, : hi - lo].T
    return diff, spec


def kernel(**inputs):
    nrm = np.asarray(inputs["normals"], np.float32)
    vd = np.asarray(inputs["view_dirs"], np.float32)
    nn = nrm / np.maximum(np.linalg.norm(nrm, axis=1, keepdims=True), EPS)
    vv = vd / np.maximum(np.linalg.norm(vd, axis=1, keepdims=True), EPS)
    dot = np.einsum("ij,ij->i", nn, vv)
    nv = int((dot > 0).sum())
    nt = max(_required_nt(nv), DEFAULT_NT)
    nc = get_nc(nt)
    shards = make_shards(inputs, nt)
    res = run_bass_kernel_spmd(nc, shards, core_ids=list(range(NCORES)))
    return gather_outputs(res.results)
